# revision 9
# baseline (speedup 1.0000x reference)
"""Cross-attention kernel for Trainium2 (Bass/Tile), 8-core SPMD. v3.

Problem: single-head cross attention over flattened 64x64 spatial positions.
  Q = Wq @ x_q                 [B,128,4096]   (bq = 0)
  K = Wk @ x_kv                [B,128,4096]   (bk = 0)
  V = Wv @ x_kv + bv           [B,128,4096]
  attn = softmax(0.25 * Q^T K) over keys      [B,4096,4096]
  out  = Wo @ (attn @ V^T)^T + bo + x_q       [B,128,64,64]

Sharding: data-parallel over batch (4 samples) x 2-way query split = 8 cores.
Each core: 2048 queries vs all 4096 keys of one sample.

Host-side algebraic folds (all exact for this problem's zero q/k biases):
  - Wq folded into the K projection:  S = x_q^T (SCALE Wq^T Wk) x_kv.
  - Wo folded into Wv:  out = attn @ (Wo Wv x_kv)^T + (Wo bv + bo) + x_q,
    using sum_k attn[q,k] = 1.
  - (Wo bv + bo) folded into the bf16 residual input.

v3 structure (vs v2): query tile = 512 so every PSUM tile is a single
bank.  PSUM = 6-buf S ring + 1 PV + 1 sum = exactly 8 banks.  The deep S
ring gives the PE ~1.7us of slack against the exp engines, so it streams
matmuls back-to-back at the full 2.4 GHz p-state (a stalling PE drops to
~2.0 GHz).  Per pair-step (256 keys x 512 queries):
  S0[k,q] = Ksb_chunk.T @ xq (PE bf16, 518cy)  -> ACT exp -> pt[:, 0:512]
  S1[k,q] = next chunk       (PE bf16)         -> DVE Schraudolph fast-exp
                                                  (u8 = A8*x+B8 is the
                                                  e4m3 bit pattern of e^x)
                                                  -> pt[:, 512:1024]
  pv  += VT_pair.T @ pt_pair (PE fp8 DoubleRow, 256-deep contraction)
  sum += ones.T @ pt_pair    (PE fp8 DR)
Tail per qtile: recip (DVE) -> broadcast matmul (PE) -> SBUF stage (ACT)
-> normalize mul (DVE) -> +residual (GPSIMD, SBUF-only engine) -> DMA out
(bf16).  Residual add and output ride idle engines to keep ACT/DVE free
for exp, which binds at ~64 chunks x ~790ns per engine.

No max-subtraction in softmax: |0.25*Q^T K| <= ~1.3 for this problem's
fixed input distribution, so exp never overflows.
"""

import sys

if "/opt/trn_rl_repo" not in sys.path:
    sys.path.insert(0, "/opt/trn_rl_repo")

import numpy as np
import ml_dtypes

B, CQ, CKV, H, W = 4, 128, 256, 64, 64
N = H * W            # 4096 positions
NH = N // 2          # 2048 queries per core
QT = 512             # query tile (free-dim of the S^T matmuls)
NQT = NH // QT       # 4 query tiles per core
KC = 128             # key chunk (partition dim of S^T)
NKC = N // KC        # 32 key chunks
SCALE = (CQ // 8) ** (-0.5)  # 0.25

# fp8 e4m3 Schraudolph: uint8 = A8*x + B8 is the e4m3 bit pattern of e^x
# (max rel err ~7%, cancelled by softmax renormalization)
SCHRAUD_A8 = 8.0 / np.log(2.0)
SCHRAUD_B8 = 55.62
# fp8 weight scales to keep quantized values in e4m3 normal range
WS_K = 256.0         # folded SCALE*Wq^T*Wk entries ~1e-3
WS_V = 64.0          # folded Wo*Wv entries ~5e-3

_cache = {}


def _build_program():
    import concourse.bass as bass  # noqa: F401
    from concourse import bacc
    import concourse.mybir as mybir
    import concourse.tile as tile

    f32 = mybir.dt.float32
    bf16 = mybir.dt.bfloat16
    fp8 = mybir.dt.float8e4
    u8 = mybir.dt.uint8
    AF = mybir.ActivationFunctionType
    ALU = mybir.AluOpType

    nc = bacc.Bacc(
        "TRN2",
        target_bir_lowering=False,
        debug=False,
        enable_asserts=False,
        num_devices=8,
    )

    # ---- DRAM I/O (per-core shapes) ----
    # wpack: cols 0:256 = wk2 (r-major pairs), cols 256:512 = wv2
    d_wpack = nc.dram_tensor("wpack", [128, 512], fp8, kind="ExternalInput").ap()
    d_xq16 = nc.dram_tensor("xq16", [CQ, NH], bf16, kind="ExternalInput").ap()
    d_xqres = nc.dram_tensor("xqres", [CQ, NH], bf16, kind="ExternalInput").ap()
    # xkv fp8, layout [c' within half (partition), (G, r, n)]
    d_xkv8 = nc.dram_tensor("xkv8", [128, 2 * N], fp8, kind="ExternalInput").ap()
    d_out = nc.dram_tensor("out", [CQ, NH], bf16, kind="ExternalOutput").ap()

    DR = mybir.MatmulPerfMode.DoubleRow

    with tile.TileContext(nc) as tc:
        with (
            tc.tile_pool(name="const", bufs=1) as cp,
            tc.tile_pool(name="big", bufs=1) as bp,
            tc.tile_pool(name="pt", bufs=4) as ptp,
            tc.tile_pool(name="misc", bufs=2) as mp,
            tc.tile_pool(name="outp", bufs=4) as op_,
            tc.tile_pool(name="mm", bufs=6, space="PSUM") as mm,
            tc.tile_pool(name="sump", bufs=1, space="PSUM") as sump,
            tc.tile_pool(name="pv", bufs=1, space="PSUM") as pvp,
        ):
            # ---- input DMAs. sync HWDGE ring: xkv8 chunks (K'/VT are the
            # first consumers), then the tail-only residual LAST. scalar
            # ring: weights, then xq16 first-tile slice, then the rest. ----
            wpack = cp.tile([128, 512], fp8, name="wpack")
            nc.scalar.dma_start(wpack, d_wpack)
            xq16 = cp.tile([128, NH], bf16, name="xq16")
            nc.scalar.dma_start(xq16[:, 0:QT], d_xq16[:, 0:QT])
            # xkv8 host layout: [p, (G, r, n)] at 512-key group granularity —
            # each group's two r-halves are column-adjacent. First two DMA
            # chunks are single groups so the K' projection starts early.
            xkv8 = cp.tile([128, 2 * N], fp8, name="xkv8")
            for lo, hi in ((0, 1), (1, 2), (2, 4), (4, 6), (6, 8)):
                sl = slice(lo * 1024, hi * 1024)
                nc.sync.dma_start(xkv8[:, sl], d_xkv8[:, sl])
            nc.scalar.dma_start(xq16[:, QT:NH], d_xq16[:, QT:NH])
            xqres = cp.tile([128, NH], bf16, name="xqres")
            nc.sync.dma_start(xqres, d_xqres)

            # pair-ones for the DoubleRow softmax-sum matmuls; 16-col halves
            # because the DR weight AP needs pair-step % 16 == 0
            ones8 = cp.tile([128, 32], fp8, name="ones8")
            nc.gpsimd.memset(ones8, 1.0)
            # broadcast-ones row carries the 1/WS_V compensation for the
            # scaled V' weights
            oner = cp.tile([1, 128], f32, name="oner")
            nc.gpsimd.memset(oner, 1.0 / WS_V)

            # DoubleRow operand views: 4D [p, r, 1, n] so the pair dim lands
            # in the ISA pattern's num_elem[2] slot (outermost, count 2)
            wk3 = wpack[:, 0:256].rearrange("p (r one m) -> p r one m", r=2, one=1)
            wv3 = wpack[:, 256:512].rearrange("p (r one m) -> p r one m", r=2, one=1)
            ones3 = ones8.rearrange("p (r one m) -> p r one m", r=2, one=1)[
                :, :, :, 0:1
            ]

            Ksb = bp.tile([128, N], bf16)
            VTsb = bp.tile([128, N], fp8)

            # per-512-key-group pair views: [p, r(stride 512), 1, n(512)]
            xkvG = [
                xkv8[:, G * 1024:(G + 1) * 1024].rearrange(
                    "p (r one n) -> p r one n", r=2, one=1
                )
                for G in range(8)
            ]

            # ---- setup: K' projection (tracks the xkv DMA). Each 512-key
            # half is one single-bank PSUM tile; PSUM->SBUF copies alternate
            # ACT/DVE. The VT projection is spread over the first tile's
            # loop steps. ----
            for gj in range(8):
                kp_ps = mm.tile([128, QT], f32, tag="s", name="kp_ps")
                nc.tensor.matmul(
                    kp_ps, wk3, xkvG[gj],
                    start=True, stop=True, perf_mode=DR,
                )
                ksl = slice(gj * 512, (gj + 1) * 512)
                if gj % 2 == 0:
                    nc.scalar.activation(
                        Ksb[:, ksl], kp_ps, AF.Identity, scale=1.0 / WS_K
                    )
                else:
                    nc.vector.tensor_scalar(
                        Ksb[:, ksl], kp_ps, 1.0 / WS_K, None, op0=ALU.mult
                    )

            def emit_vt(G, eng):
                vt_ps = mm.tile([128, QT], f32, tag="s", name="vt_ps")
                for j in range(4):
                    nc.tensor.matmul(
                        vt_ps[:, j * 128:(j + 1) * 128],
                        xkvG[G][:, :, :, j * KC:(j + 1) * KC], wv3,
                        start=True, stop=True, perf_mode=DR,
                    )
                if eng == "act":
                    nc.scalar.activation(
                        VTsb[:, G * 512:(G + 1) * 512], vt_ps, AF.Identity
                    )
                else:
                    nc.vector.tensor_copy(VTsb[:, G * 512:(G + 1) * 512], vt_ps)

            # VT group G (512 keys) feeds the PV of pair-steps 2G/2G+1 (at
            # loop steps 2G+2/2G+3); emit at step 2G-1, alternating the
            # PSUM->SBUF copy engine
            VT_SCHED = {
                0: (0, "dve"), 1: (1, "act"), 3: (2, "dve"), 5: (3, "act"),
                7: (4, "dve"), 9: (5, "act"), 11: (6, "dve"), 13: (7, "act"),
            }

            # ---- main attention loop ----
            NPAIR = NKC // 2
            LEAD = 2
            for qt in range(NQT):
                qsl = slice(qt * QT, (qt + 1) * QT)
                pv_ps = sum_ps = None
                pts = {}
                for step in range(NPAIR + LEAD):
                    if step < NPAIR:
                        pt = ptp.tile([128, 1024], fp8, tag="pt", name="pt")
                        pts[step] = pt
                        for half, kc in enumerate((2 * step, 2 * step + 1)):
                            ksl = slice(kc * KC, (kc + 1) * KC)
                            s_ps = mm.tile([128, QT], f32, tag="s", name="s_ps")
                            nc.tensor.matmul(
                                s_ps, Ksb[:, ksl], xq16[:, qsl],
                                start=True, stop=True,
                            )
                            if half == 0:
                                nc.scalar.activation(
                                    pt[:, 0:512], s_ps, AF.Exp
                                )
                            else:
                                nc.vector.tensor_scalar(
                                    pt[:, 512:1024].bitcast(u8), s_ps,
                                    SCHRAUD_A8, SCHRAUD_B8,
                                    op0=ALU.mult, op1=ALU.add,
                                )
                        if qt == 0 and step in VT_SCHED:
                            emit_vt(*VT_SCHED[step])
                    if step == LEAD:
                        pv_ps = pvp.tile([128, QT], f32, tag="pv", name="pv_ps")
                        sum_ps = sump.tile([1, QT], f32, tag="sum", name="sum_ps")
                    if step >= LEAD:
                        p = step - LEAD
                        pt = pts.pop(p)
                        vt3 = VTsb[:, p * 256:(p + 1) * 256].rearrange(
                            "q (r one m) -> q r one m", r=2, one=1
                        )
                        pt3 = pt.rearrange(
                            "q (r one n) -> q r one n", r=2, one=1
                        )
                        nc.tensor.matmul(
                            pv_ps, vt3, pt3,
                            start=(p == 0), stop=(p == NPAIR - 1),
                            perf_mode=DR,
                        )
                        nc.tensor.matmul(
                            sum_ps, ones3, pt3,
                            start=(p == 0), stop=(p == NPAIR - 1),
                            perf_mode=DR,
                        )
                # tail: recip (DVE) -> broadcast matmul (PE, borrows an S
                # ring slot so the sum/pv banks stay free for the next
                # qtile) -> SBUF stage (ACT) -> normalize (DVE) ->
                # +residual (GPSIMD) -> DMA (bf16).
                recip = mp.tile([1, QT], f32, name="recip")
                nc.vector.reciprocal_approx_fast(recip, sum_ps)
                bc_ps = mm.tile([128, QT], f32, tag="s", name="bc_ps")
                nc.tensor.matmul(bc_ps, oner, recip, start=True, stop=True)
                bc_sb = mp.tile([128, QT], f32, name="bc_sb")
                nc.scalar.copy(bc_sb, bc_ps)
                outn = op_.tile([128, QT], f32, name="outn")
                nc.vector.scalar_tensor_tensor(
                    outn, pv_ps, 1.0, bc_sb,
                    op0=ALU.mult, op1=ALU.mult,
                )
                outf = op_.tile([128, QT], bf16, name="outf")
                nc.gpsimd.tensor_add(outf, outn, xqres[:, qsl])
                eng = nc.sync if qt % 2 == 0 else nc.scalar
                eng.dma_start(d_out[:, qsl], outf)

    nc.compile()
    return nc


def _get_program():
    if "nc" not in _cache:
        _cache["nc"] = _build_program()
    return _cache["nc"]


def _make_in_maps(x_q, x_kv, Wq, bq, Wk, bk, Wv, bv, Wo, bo):
    bf16 = ml_dtypes.bfloat16
    f32 = np.float32
    fp8 = ml_dtypes.float8_e4m3fn

    x_q = np.asarray(x_q, dtype=f32).reshape(B, CQ, N)
    x_kv = np.asarray(x_kv, dtype=f32).reshape(B, CKV, N)
    Wq = np.asarray(Wq, dtype=f32)
    Wk = np.asarray(Wk, dtype=f32)
    Wv = np.asarray(Wv, dtype=f32)
    Wo = np.asarray(Wo, dtype=f32)
    bq = np.asarray(bq, dtype=f32)
    bv = np.asarray(bv, dtype=f32)
    bo = np.asarray(bo, dtype=f32)

    # the Wq fold drops the per-key bias term bq^T Wk x_kv; only valid when
    # bq == 0 (true for this problem). bk only contributes softmax-invariant
    # per-query terms and drops for any bk.
    assert np.all(bq == 0.0), "Wq fold requires bq == 0"

    # host-side algebraic folds
    W2 = (Wq.T @ Wk) * SCALE           # [128, 256]
    Wv2 = Wo @ Wv                      # [128, 256]
    b_final = Wo @ bv + bo             # [128]
    w2T = W2.T * WS_K                  # [256, 128], scaled for fp8 range
    wvT = Wv2.T * WS_V                 # [256, 128], scaled for fp8 range
    # r-major pair layout for DoubleRow: [c' within half, (half, col)]
    wk8 = np.stack([w2T[:128], w2T[128:]], axis=1).reshape(128, 256)
    wv8 = np.stack([wvT[:128], wvT[128:]], axis=1).reshape(128, 256)
    wpack = np.concatenate([wk8, wv8], axis=1).astype(fp8)  # [128, 512]

    in_maps = []
    for core in range(8):
        b, half = divmod(core, 2)
        sl = slice(half * NH, (half + 1) * NH)
        # [p, (G, r, n)]: 512-key group G holds both c'-halves adjacently
        xkv8 = (
            x_kv[b].reshape(2, 128, 8, 512).transpose(1, 2, 0, 3)
            .reshape(128, 2 * N)
        )
        in_maps.append(
            {
                "xq16": x_q[b][:, sl].astype(bf16),
                "xqres": (x_q[b][:, sl] + b_final[:, None]).astype(bf16),
                "xkv8": xkv8.astype(fp8),
                "wpack": np.ascontiguousarray(wpack),
            }
        )
    return in_maps


def _assemble(results):
    out = np.empty((B, CQ, N), dtype=np.float32)
    for core in range(8):
        b, half = divmod(core, 2)
        out[b][:, half * NH:(half + 1) * NH] = results[core]["out"].astype(
            np.float32
        )
    return out.reshape(B, CQ, H, W)


def run_raw(in_maps, trace=False, core_ids_override=None, **kwargs):
    from concourse.bass_utils import run_bass_kernel_spmd

    nc = _get_program()
    core_ids = core_ids_override or list(range(8))
    return run_bass_kernel_spmd(
        nc, in_maps, core_ids=core_ids, trace=trace, **kwargs
    )


def kernel(**inputs) -> np.ndarray:
    in_maps = _make_in_maps(**inputs)
    res = run_raw(in_maps)
    return _assemble(res.results)


def kernel_profiled(**inputs):
    """Returns (output, BassKernelResults-with-trace)."""
    in_maps = _make_in_maps(**inputs)
    res = run_raw(in_maps, trace=True)
    return _assemble(res.results), res


# revision 10
# speedup vs baseline: 1.1783x; 1.1783x over previous
"""Cross-attention kernel for Trainium2 (Bass/Tile), 8-core SPMD. v3.

Problem: single-head cross attention over flattened 64x64 spatial positions.
  Q = Wq @ x_q                 [B,128,4096]   (bq = 0)
  K = Wk @ x_kv                [B,128,4096]   (bk = 0)
  V = Wv @ x_kv + bv           [B,128,4096]
  attn = softmax(0.25 * Q^T K) over keys      [B,4096,4096]
  out  = Wo @ (attn @ V^T)^T + bo + x_q       [B,128,64,64]

Sharding: data-parallel over batch (4 samples) x 2-way query split = 8 cores.
Each core: 2048 queries vs all 4096 keys of one sample.

Host-side algebraic folds (all exact for this problem's zero q/k biases):
  - Wq folded into the K projection:  S = x_q^T (SCALE Wq^T Wk) x_kv.
  - Wo folded into Wv:  out = attn @ (Wo Wv x_kv)^T + (Wo bv + bo) + x_q,
    using sum_k attn[q,k] = 1.
  - (Wo bv + bo) folded into the bf16 residual input.

v3 structure (vs v2): query tile = 512 so every PSUM tile is a single
bank.  PSUM = 6-buf S ring + 1 PV + 1 sum = exactly 8 banks.  The deep S
ring gives the PE ~1.7us of slack against the exp engines, so it streams
matmuls back-to-back at the full 2.4 GHz p-state (a stalling PE drops to
~2.0 GHz).  Per pair-step (256 keys x 512 queries):
  S0[k,q] = Ksb_chunk.T @ xq (PE bf16, 518cy)  -> ACT exp -> pt[:, 0:512]
  S1[k,q] = next chunk       (PE bf16)         -> DVE Schraudolph fast-exp
                                                  (u8 = A8*x+B8 is the
                                                  e4m3 bit pattern of e^x)
                                                  -> pt[:, 512:1024]
  pv  += VT_pair.T @ pt_pair (PE fp8 DoubleRow, 256-deep contraction)
  sum += ones.T @ pt_pair    (PE fp8 DR)
Tail per qtile: recip (DVE) -> broadcast matmul (PE) -> SBUF stage (ACT)
-> normalize mul (DVE) -> +residual (GPSIMD, SBUF-only engine) -> DMA out
(bf16).  Residual add and output ride idle engines to keep ACT/DVE free
for exp, which binds at ~64 chunks x ~790ns per engine.

No max-subtraction in softmax: |0.25*Q^T K| <= ~1.3 for this problem's
fixed input distribution, so exp never overflows.
"""

import sys

if "/opt/trn_rl_repo" not in sys.path:
    sys.path.insert(0, "/opt/trn_rl_repo")

import numpy as np
import ml_dtypes

B, CQ, CKV, H, W = 4, 128, 256, 64, 64
N = H * W            # 4096 positions
NH = N // 2          # 2048 queries per core
QT = 512             # query tile (free-dim of the S^T matmuls)
NQT = NH // QT       # 4 query tiles per core
KC = 128             # key chunk (partition dim of S^T)
NKC = N // KC        # 32 key chunks
SCALE = (CQ // 8) ** (-0.5)  # 0.25

# fp8 e4m3 Schraudolph: uint8 = A8*x + B8 is the e4m3 bit pattern of e^x
# (max rel err ~7%, cancelled by softmax renormalization)
SCHRAUD_A8 = 8.0 / np.log(2.0)
SCHRAUD_B8 = 55.62
# fp8 weight scales to keep quantized values in e4m3 normal range
WS_K = 256.0         # folded SCALE*Wq^T*Wk entries ~1e-3
WS_V = 64.0          # folded Wo*Wv entries ~5e-3

_cache = {}


def _build_program():
    import concourse.bass as bass  # noqa: F401
    from concourse import bacc
    import concourse.mybir as mybir
    import concourse.tile as tile

    f32 = mybir.dt.float32
    bf16 = mybir.dt.bfloat16
    fp8 = mybir.dt.float8e4
    u8 = mybir.dt.uint8
    AF = mybir.ActivationFunctionType
    ALU = mybir.AluOpType

    nc = bacc.Bacc(
        "TRN2",
        target_bir_lowering=False,
        debug=False,
        enable_asserts=False,
        num_devices=8,
    )

    # ---- DRAM I/O (per-core shapes) ----
    # wpack: cols 0:256 = wk2 (r-major pairs), cols 256:512 = wv2
    d_wpack = nc.dram_tensor("wpack", [128, 512], fp8, kind="ExternalInput").ap()
    d_xq16 = nc.dram_tensor("xq16", [CQ, NH], bf16, kind="ExternalInput").ap()
    d_xqres = nc.dram_tensor("xqres", [CQ, NH], bf16, kind="ExternalInput").ap()
    # xkv fp8, layout [c' within half (partition), (G, r, n)]
    d_xkv8 = nc.dram_tensor("xkv8", [128, 2 * N], fp8, kind="ExternalInput").ap()
    d_out = nc.dram_tensor("out", [CQ, NH], bf16, kind="ExternalOutput").ap()

    DR = mybir.MatmulPerfMode.DoubleRow

    with tile.TileContext(nc) as tc:
        with (
            tc.tile_pool(name="const", bufs=1) as cp,
            tc.tile_pool(name="big", bufs=1) as bp,
            tc.tile_pool(name="pt", bufs=4) as ptp,
            tc.tile_pool(name="misc", bufs=2) as mp,
            tc.tile_pool(name="outp", bufs=4) as op_,
            tc.tile_pool(name="mm", bufs=6, space="PSUM") as mm,
            tc.tile_pool(name="sump", bufs=1, space="PSUM") as sump,
            tc.tile_pool(name="pv", bufs=1, space="PSUM") as pvp,
        ):
            # ---- input DMAs. sync HWDGE ring: xkv8 chunks (K'/VT are the
            # first consumers), then the tail-only residual LAST. scalar
            # ring: weights, then xq16 first-tile slice, then the rest. ----
            wpack = cp.tile([128, 512], fp8, name="wpack")
            nc.scalar.dma_start(wpack, d_wpack)
            xq16 = cp.tile([128, NH], bf16, name="xq16")
            nc.scalar.dma_start(xq16[:, 0:QT], d_xq16[:, 0:QT])
            # xkv8 host layout: [p, (G, r, n)] at 512-key group granularity —
            # each group's two r-halves are column-adjacent. First two DMA
            # chunks are single groups so the K' projection starts early.
            xkv8 = cp.tile([128, 2 * N], fp8, name="xkv8")
            for lo, hi in ((0, 1), (1, 2), (2, 4), (4, 6), (6, 8)):
                sl = slice(lo * 1024, hi * 1024)
                nc.sync.dma_start(xkv8[:, sl], d_xkv8[:, sl])
            nc.scalar.dma_start(xq16[:, QT:NH], d_xq16[:, QT:NH])
            xqres = cp.tile([128, NH], bf16, name="xqres")
            nc.sync.dma_start(xqres, d_xqres)

            # pair-ones for the DoubleRow softmax-sum matmuls; 16-col halves
            # because the DR weight AP needs pair-step % 16 == 0
            ones8 = cp.tile([128, 32], fp8, name="ones8")
            nc.gpsimd.memset(ones8, 1.0)
            # broadcast-ones row carries the 1/WS_V compensation for the
            # scaled V' weights
            oner = cp.tile([1, 128], f32, name="oner")
            nc.gpsimd.memset(oner, 1.0 / WS_V)

            # DoubleRow operand views: 4D [p, r, 1, n] so the pair dim lands
            # in the ISA pattern's num_elem[2] slot (outermost, count 2)
            wk3 = wpack[:, 0:256].rearrange("p (r one m) -> p r one m", r=2, one=1)
            wv3 = wpack[:, 256:512].rearrange("p (r one m) -> p r one m", r=2, one=1)
            ones3 = ones8.rearrange("p (r one m) -> p r one m", r=2, one=1)[
                :, :, :, 0:1
            ]

            Ksb = bp.tile([128, N], bf16)
            VTsb = bp.tile([128, N], fp8)

            # per-512-key-group pair views: [p, r(stride 512), 1, n(512)]
            xkvG = [
                xkv8[:, G * 1024:(G + 1) * 1024].rearrange(
                    "p (r one n) -> p r one n", r=2, one=1
                )
                for G in range(8)
            ]

            # ---- setup: K' projection (tracks the xkv DMA). Each 512-key
            # half is one single-bank PSUM tile; PSUM->SBUF copies alternate
            # ACT/DVE. The VT projection is spread over the first tile's
            # loop steps. ----
            for gj in range(8):
                kp_ps = mm.tile([128, QT], f32, tag="s", name="kp_ps")
                nc.tensor.matmul(
                    kp_ps, wk3, xkvG[gj],
                    start=True, stop=True, perf_mode=DR,
                )
                ksl = slice(gj * 512, (gj + 1) * 512)
                if gj % 2 == 0:
                    nc.scalar.activation(
                        Ksb[:, ksl], kp_ps, AF.Identity, scale=1.0 / WS_K
                    )
                else:
                    nc.vector.tensor_scalar(
                        Ksb[:, ksl], kp_ps, 1.0 / WS_K, None, op0=ALU.mult
                    )

            def emit_vt(G, eng):
                vt_ps = mm.tile([128, QT], f32, tag="s", name="vt_ps")
                for j in range(4):
                    nc.tensor.matmul(
                        vt_ps[:, j * 128:(j + 1) * 128],
                        xkvG[G][:, :, :, j * KC:(j + 1) * KC], wv3,
                        start=True, stop=True, perf_mode=DR,
                    )
                if eng == "act":
                    nc.scalar.activation(
                        VTsb[:, G * 512:(G + 1) * 512], vt_ps, AF.Identity
                    )
                else:
                    nc.vector.tensor_copy(VTsb[:, G * 512:(G + 1) * 512], vt_ps)

            # VT group G (512 keys) feeds the PV of pair-steps 2G/2G+1 (at
            # loop steps 2G+2/2G+3); emit at step 2G-1, alternating the
            # PSUM->SBUF copy engine
            VT_SCHED = {
                0: (0, "dve"), 1: (1, "act"), 3: (2, "dve"), 5: (3, "act"),
                7: (4, "dve"), 9: (5, "act"), 11: (6, "dve"), 13: (7, "act"),
            }

            # ---- main attention loop ----
            NPAIR = NKC // 2
            LEAD = 2
            for qt in range(NQT):
                qsl = slice(qt * QT, (qt + 1) * QT)
                pv_ps = sum_ps = None
                pts = {}
                for step in range(NPAIR + LEAD):
                    if step < NPAIR:
                        pt = ptp.tile([128, 1024], fp8, tag="pt", name="pt")
                        pts[step] = pt
                        for half, kc in enumerate((2 * step, 2 * step + 1)):
                            ksl = slice(kc * KC, (kc + 1) * KC)
                            s_ps = mm.tile([128, QT], f32, tag="s", name="s_ps")
                            nc.tensor.matmul(
                                s_ps, Ksb[:, ksl], xq16[:, qsl],
                                start=True, stop=True,
                            )
                            if half == 0:
                                nc.scalar.activation(
                                    pt[:, 0:512], s_ps, AF.Exp
                                )
                            else:
                                nc.vector.tensor_scalar(
                                    pt[:, 512:1024].bitcast(u8), s_ps,
                                    SCHRAUD_A8, SCHRAUD_B8,
                                    op0=ALU.mult, op1=ALU.add,
                                )
                        if qt == 0 and step in VT_SCHED:
                            emit_vt(*VT_SCHED[step])
                    if step == LEAD:
                        pv_ps = pvp.tile([128, QT], f32, tag="pv", name="pv_ps")
                        sum_ps = sump.tile([1, QT], f32, tag="sum", name="sum_ps")
                    if step >= LEAD:
                        p = step - LEAD
                        pt = pts.pop(p)
                        vt3 = VTsb[:, p * 256:(p + 1) * 256].rearrange(
                            "q (r one m) -> q r one m", r=2, one=1
                        )
                        pt3 = pt.rearrange(
                            "q (r one n) -> q r one n", r=2, one=1
                        )
                        nc.tensor.matmul(
                            pv_ps, vt3, pt3,
                            start=(p == 0), stop=(p == NPAIR - 1),
                            perf_mode=DR,
                        )
                        nc.tensor.matmul(
                            sum_ps, ones3, pt3,
                            start=(p == 0), stop=(p == NPAIR - 1),
                            perf_mode=DR,
                        )
                # tail: recip (DVE) -> broadcast matmul (PE, borrows an S
                # ring slot so the sum/pv banks stay free for the next
                # qtile) -> SBUF stage (ACT) -> normalize (DVE) ->
                # +residual (GPSIMD) -> DMA (bf16).
                recip = mp.tile([1, QT], f32, name="recip")
                nc.vector.reciprocal_approx_fast(recip, sum_ps)
                bc_ps = sump.tile([128, QT], f32, tag="sum", name="bc_ps")
                nc.tensor.matmul(bc_ps, oner, recip, start=True, stop=True)
                bc_sb = mp.tile([128, QT], f32, name="bc_sb")
                nc.scalar.copy(bc_sb, bc_ps)
                outn = op_.tile([128, QT], f32, name="outn")
                nc.vector.scalar_tensor_tensor(
                    outn, pv_ps, 1.0, bc_sb,
                    op0=ALU.mult, op1=ALU.mult,
                )
                outf = op_.tile([128, QT], bf16, name="outf")
                nc.gpsimd.tensor_add(outf, outn, xqres[:, qsl])
                eng = nc.sync if qt % 2 == 0 else nc.scalar
                eng.dma_start(d_out[:, qsl], outf)

    nc.compile()
    return nc


def _get_program():
    if "nc" not in _cache:
        _cache["nc"] = _build_program()
    return _cache["nc"]


def _make_in_maps(x_q, x_kv, Wq, bq, Wk, bk, Wv, bv, Wo, bo):
    bf16 = ml_dtypes.bfloat16
    f32 = np.float32
    fp8 = ml_dtypes.float8_e4m3fn

    x_q = np.asarray(x_q, dtype=f32).reshape(B, CQ, N)
    x_kv = np.asarray(x_kv, dtype=f32).reshape(B, CKV, N)
    Wq = np.asarray(Wq, dtype=f32)
    Wk = np.asarray(Wk, dtype=f32)
    Wv = np.asarray(Wv, dtype=f32)
    Wo = np.asarray(Wo, dtype=f32)
    bq = np.asarray(bq, dtype=f32)
    bv = np.asarray(bv, dtype=f32)
    bo = np.asarray(bo, dtype=f32)

    # the Wq fold drops the per-key bias term bq^T Wk x_kv; only valid when
    # bq == 0 (true for this problem). bk only contributes softmax-invariant
    # per-query terms and drops for any bk.
    assert np.all(bq == 0.0), "Wq fold requires bq == 0"

    # host-side algebraic folds
    W2 = (Wq.T @ Wk) * SCALE           # [128, 256]
    Wv2 = Wo @ Wv                      # [128, 256]
    b_final = Wo @ bv + bo             # [128]
    w2T = W2.T * WS_K                  # [256, 128], scaled for fp8 range
    wvT = Wv2.T * WS_V                 # [256, 128], scaled for fp8 range
    # r-major pair layout for DoubleRow: [c' within half, (half, col)]
    wk8 = np.stack([w2T[:128], w2T[128:]], axis=1).reshape(128, 256)
    wv8 = np.stack([wvT[:128], wvT[128:]], axis=1).reshape(128, 256)
    wpack = np.concatenate([wk8, wv8], axis=1).astype(fp8)  # [128, 512]

    in_maps = []
    for core in range(8):
        b, half = divmod(core, 2)
        sl = slice(half * NH, (half + 1) * NH)
        # [p, (G, r, n)]: 512-key group G holds both c'-halves adjacently
        xkv8 = (
            x_kv[b].reshape(2, 128, 8, 512).transpose(1, 2, 0, 3)
            .reshape(128, 2 * N)
        )
        in_maps.append(
            {
                "xq16": x_q[b][:, sl].astype(bf16),
                "xqres": (x_q[b][:, sl] + b_final[:, None]).astype(bf16),
                "xkv8": xkv8.astype(fp8),
                "wpack": np.ascontiguousarray(wpack),
            }
        )
    return in_maps


def _assemble(results):
    out = np.empty((B, CQ, N), dtype=np.float32)
    for core in range(8):
        b, half = divmod(core, 2)
        out[b][:, half * NH:(half + 1) * NH] = results[core]["out"].astype(
            np.float32
        )
    return out.reshape(B, CQ, H, W)


def run_raw(in_maps, trace=False, core_ids_override=None, **kwargs):
    from concourse.bass_utils import run_bass_kernel_spmd

    nc = _get_program()
    core_ids = core_ids_override or list(range(8))
    return run_bass_kernel_spmd(
        nc, in_maps, core_ids=core_ids, trace=trace, **kwargs
    )


def kernel(**inputs) -> np.ndarray:
    in_maps = _make_in_maps(**inputs)
    res = run_raw(in_maps)
    return _assemble(res.results)


def kernel_profiled(**inputs):
    """Returns (output, BassKernelResults-with-trace)."""
    in_maps = _make_in_maps(**inputs)
    res = run_raw(in_maps, trace=True)
    return _assemble(res.results), res
